# revision 21
# baseline (speedup 1.0000x reference)
"""Bahdanau additive attention on 8 Trainium2 NeuronCores (Bass/Tile).

Math: context, attention = softmax_k(sum_h We_h * tanh(qproj[q,h] + kproj[k,h]) + be)
per batch.  be shifts all scores equally -> softmax-invariant -> dropped.

Key trick: tanh(a+b) ~= sum_m c_m sin(w_m (a+b))
                      = sum_m c_m [sin(w_m a) cos(w_m b) + cos(w_m a) sin(w_m b)]
with quarter-wave harmonics w_m = (2m-1) pi/(2L).  This factorizes the
(Lq x Lk x H) tanh into per-side sin/cos features contracted by the Tensor
engine.  The ScalarE Sin LUT is only valid on [-pi,pi]: angles are
range-reduced exactly on the Vector engine with the magic-number rounding
trick (skipped for the lowest harmonics whose angles provably stay in
range); cos(y) = sin(pi/2 - |y|) via an ACT Abs pass + Sin(scale=-1,
bias=pi/2).  |We|*c_m folds into the a-side features; sign(We) folds into
the projections (tanh is odd).  Matmul operands are fp16 (single-pass PE;
fp32 is LOW_HIGH double-pass); accumulation is fp32 in PSUM and the
range-reduction arithmetic is fp32.  Transposed/cast operand layouts and
the tiny We-derived vectors are prepared host-side during input sharding.

Sharding: core c <- (batch b=c//2, query-half qh=c%2); each core computes a
[256 q x 512 k] attention block + [256 x 256] context block independently.
"""
from contextlib import ExitStack

import numpy as np

import concourse.bass as bass
import concourse.mybir as mybir
import concourse.tile as tile
from concourse import bacc
from concourse.bass_utils import run_bass_kernel_spmd

F32 = mybir.dt.float32
F16 = mybir.dt.float16
AF = mybir.ActivationFunctionType
ALU = mybir.AluOpType

# ---- problem shape (hardcoded per contract) ----
B, LQ, LK, H = 4, 512, 512, 256
LQC = LQ // 2          # per-core queries = 256
HT = H // 128          # h tiles = 2
KT = LK // 128         # k tiles = 4
QB = LQC // 128        # q blocks per core = 2

# ---- sine approximation of tanh ----
M_TERMS = 10
GROUP = 2              # m's per feature group
L_FIT = 6.5
MAGIC = float(np.float32(1.5 * 2 ** 23))
TWO_PI = float(np.float32(2 * np.pi))
PI = float(np.pi)
T_SCALE = 1.0 / (4.0 * L_FIT)    # turns per unit: w_base/(2*pi) = 1/(4L)
# |2pi*mp*t| stays inside [-pi,pi] for |proj|<=4.3 -> no range reduction.
FOLD_FREE = {1.0, 3.0}
# |2pi*mp*t| stays inside [-3pi,3pi] -> one add_range_wrap suffices.
WRAP_FOLD = {5.0, 7.0}
# |2pi*mp*t| stays inside [-pi/2,pi/2] -> cos needs no abs (pi/2 - y in range).
ABS_FREE = {1.0}

NA = HT * LQC   # 512  (a-side feature width per m)
NB = HT * LK    # 1024 (b-side feature width per m)


def _fit_coeffs():
    x = np.linspace(0, L_FIT, 6001)
    mult = 2 * np.arange(1, M_TERMS + 1) - 1
    w = mult * np.pi / (2 * L_FIT)
    A = np.sin(np.outer(x, w))
    c, *_ = np.linalg.lstsq(A, np.tanh(x), rcond=None)
    return [float(m) for m in mult], [float(v) for v in c]


MULT, COEF = _fit_coeffs()

_PROGRAM = None


def _build_program():
    nc = bacc.Bacc("TRN2", target_bir_lowering=False, debug=False, num_devices=8)

    # Pre-transposed fp16 matmul operands (prepared host-side):
    qT_d = nc.dram_tensor("queryT", [H, LQC], F16, kind="ExternalInput").ap()
    vT_d = nc.dram_tensor("valuesT", [H, LK], F16, kind="ExternalInput").ap()
    v16_d = nc.dram_tensor("values16", [LK, H], F16, kind="ExternalInput").ap()
    wqT_d = nc.dram_tensor("WqT", [H, H], F16, kind="ExternalInput").ap()
    wkT_d = nc.dram_tensor("WkT", [H, H], F16, kind="ExternalInput").ap()
    # Small fp32 vectors (bias / We-derived), in [128, x] on-chip layout:
    bqT_d = nc.dram_tensor("bqT", [128, HT], F32, kind="ExternalInput").ap()
    bkT_d = nc.dram_tensor("bkT", [128, HT], F32, kind="ExternalInput").ap()
    sgn_d = nc.dram_tensor("sgn_sc", [128, HT], F32, kind="ExternalInput").ap()
    wc_d = nc.dram_tensor("wc", [128, HT * M_TERMS], F32, kind="ExternalInput").ap()
    attn_d = nc.dram_tensor("attn", [LQC, LK], F16, kind="ExternalOutput").ap()
    ctx_d = nc.dram_tensor("ctxout", [LQC, H], F32, kind="ExternalOutput").ap()

    with tile.TileContext(nc) as tc, ExitStack() as ctx:
        persist = ctx.enter_context(tc.tile_pool(name="persist", bufs=1))
        work = ctx.enter_context(tc.tile_pool(name="work", bufs=3))
        xpool = ctx.enter_context(tc.tile_pool(name="xpool", bufs=2))
        fpool = ctx.enter_context(tc.tile_pool(name="fpool", bufs=1))
        ps_small = ctx.enter_context(tc.tile_pool(name="ps_small", bufs=2, space="PSUM"))
        ps_proj = ctx.enter_context(tc.tile_pool(name="ps_proj", bufs=2, space="PSUM"))
        ps_score = ctx.enter_context(tc.tile_pool(name="ps_score", bufs=1, space="PSUM"))

        from concourse.masks import make_identity
        ident16 = persist.tile([128, 128], F16, tag="ident16", name="ident16")
        make_identity(nc, ident16)
        half_pi = persist.tile([128, 1], F32, tag="half_pi", name="half_pi")
        nc.vector.memset(half_pi[:], PI / 2)

        # ---------- loads ----------
        valuesT = persist.tile([128, HT * LK], F16, tag="valuesT", name="valuesT")
        wkTt = persist.tile([128, HT * H], F16, tag="wkTt", name="wkTt")
        queryT = persist.tile([128, HT * LQC], F16, tag="queryT", name="queryT")
        wqTt = persist.tile([128, HT * H], F16, tag="wqTt", name="wqTt")
        vn16 = persist.tile([128, KT * H], F16, tag="vn16", name="vn16")
        nc.sync.dma_start(valuesT[:], vT_d.rearrange("(t p) k -> p t k", p=128))
        nc.sync.dma_start(wkTt[:], wkT_d.rearrange("(t p) o -> p t o", p=128))
        nc.sync.dma_start(queryT[:], qT_d.rearrange("(t p) i -> p t i", p=128))
        nc.sync.dma_start(wqTt[:], wqT_d.rearrange("(t p) o -> p t o", p=128))
        nc.scalar.dma_start(vn16[:], v16_d.rearrange("(i p) h -> p i h", p=128))
        bqT = persist.tile([128, HT], F32, tag="bqT", name="bqT")
        bkT = persist.tile([128, HT], F32, tag="bkT", name="bkT")
        sgn_sc = persist.tile([128, HT], F32, tag="sgn_sc", name="sgn_sc")
        wc = persist.tile([128, HT * M_TERMS], F32, tag="wc", name="wc")
        nc.scalar.dma_start(bkT[:], bkT_d)
        nc.scalar.dma_start(sgn_sc[:], sgn_d)
        nc.scalar.dma_start(bqT[:], bqT_d)
        nc.scalar.dma_start(wc[:], wc_d)

        # ---------- projections -> turn grids (fp32) ----------
        tk = persist.tile([128, NB], F32, tag="tk", name="tk")   # [t0 512 | t1 512]
        tq = persist.tile([128, NA], F32, tag="tq", name="tq")   # [t0 256 | t1 256]
        for ot in range(HT):
            psk = ps_proj.tile([128, LK], F32, tag="pj", name="psk")
            for ht in range(HT):
                nc.tensor.matmul(psk[:], wkTt[:, ht * H + ot * 128: ht * H + (ot + 1) * 128],
                                 valuesT[:, ht * LK:(ht + 1) * LK],
                                 start=(ht == 0), stop=(ht == HT - 1))
            tmp = work.tile([128, LK], F32, tag="projk", name="projk")
            nc.scalar.activation(tmp[:], psk[:], AF.Identity, bias=bkT[:, ot:ot + 1])
            nc.vector.tensor_scalar(tk[:, ot * LK:(ot + 1) * LK], tmp[:],
                                    sgn_sc[:, ot:ot + 1], None, ALU.mult)
        for ot in range(HT):
            psq = ps_proj.tile([128, LK], F32, tag="pj", name="psq")[:, :LQC]
            for ht in range(HT):
                nc.tensor.matmul(psq[:], wqTt[:, ht * H + ot * 128: ht * H + (ot + 1) * 128],
                                 queryT[:, ht * LQC:(ht + 1) * LQC],
                                 start=(ht == 0), stop=(ht == HT - 1))
            tmp = work.tile([128, LQC], F32, tag="projq", name="projq")
            nc.scalar.activation(tmp[:], psq[:], AF.Identity, bias=bqT[:, ot:ot + 1])
            nc.vector.tensor_scalar(tq[:, ot * LQC:(ot + 1) * LQC], tmp[:],
                                    sgn_sc[:, ot:ot + 1], None, ALU.mult)

        # ---------- fold helper ----------
        def fold_side(src, n, mp, xs_dst):
            """xs_dst <- 2pi*mp*src - 2pi*round(mp*src), in [-pi, pi]."""
            if mp in FOLD_FREE:
                nc.vector.tensor_scalar(xs_dst, src[:], TWO_PI * mp, None, ALU.mult)
                return
            if mp in WRAP_FOLD:
                rad = xpool.tile([128, n], F32, tag=f"rad{n}", name=f"rad{n}")
                nc.vector.tensor_scalar(rad[:], src[:], TWO_PI * mp, None, ALU.mult)
                nc.vector.add_range_wrap(xs_dst, rad[:], shift=0.0, bound=PI,
                                         period=TWO_PI)
                return
            ys = xpool.tile([128, n], F32, tag=f"ys{n}", name=f"ys{n}")
            nc.vector.tensor_scalar(ys[:], src[:], mp, MAGIC, ALU.mult, ALU.add)
            z = xpool.tile([128, n], F32, tag=f"z{n}", name=f"z{n}")
            nc.vector.tensor_scalar(z[:], ys[:], MAGIC, -TWO_PI,
                                    ALU.subtract, ALU.mult)
            nc.vector.scalar_tensor_tensor(xs_dst, in0=src[:], scalar=TWO_PI * mp,
                                           in1=z[:], op0=ALU.mult, op1=ALU.add)

        # ---------- feature groups ----------
        ps_sc = [ps_score.tile([128, LK], F32, tag=f"scores{qb}", name=f"scores{qb}")
                 for qb in range(QB)]

        n_groups = M_TERMS // GROUP
        feats = []   # per group: (fb_s, fb_c, fa_s, fa_c)
        for g in range(n_groups):
            ms = [g * GROUP + jl for jl in range(GROUP)]
            xs_b = xpool.tile([128, GROUP * NB], F32, tag="xs_b", name="xs_b")
            xs_a = xpool.tile([128, GROUP * NA], F32, tag="xs_a", name="xs_a")
            for jl, j in enumerate(ms):
                mp = MULT[j]
                fold_side(tk, NB, mp, xs_b[:, jl * NB:(jl + 1) * NB])
                fold_side(tq, NA, mp, xs_a[:, jl * NA:(jl + 1) * NA])
            all_abs_free = all(MULT[j] in ABS_FREE for j in ms)
            fb_s = fpool.tile([128, GROUP * NB], F16, tag=f"fb_s{g}", name=f"fb_s{g}")
            fb_c = fpool.tile([128, GROUP * NB], F16, tag=f"fb_c{g}", name=f"fb_c{g}")
            fa_s = fpool.tile([128, GROUP * NA], F16, tag=f"fa_s{g}", name=f"fa_s{g}")
            fa_c = fpool.tile([128, GROUP * NA], F16, tag=f"fa_c{g}", name=f"fa_c{g}")
            nc.scalar.activation(fb_s[:], xs_b[:], AF.Sin)
            nc.scalar.activation(fa_s[:], xs_a[:], AF.Sin)
            if all_abs_free:
                nc.scalar.activation(fb_c[:], xs_b[:], AF.Sin, scale=-1.0,
                                     bias=half_pi[:, 0:1])
                nc.scalar.activation(fa_c[:], xs_a[:], AF.Sin, scale=-1.0,
                                     bias=half_pi[:, 0:1])
            elif g == 2:
                # balance: this group's cos-prep on the Vector engine
                xabs_b = xpool.tile([128, GROUP * NB], F32, tag="xabs_b", name="xabs_b")
                xabs_a = xpool.tile([128, GROUP * NA], F32, tag="xabs_a", name="xabs_a")
                nc.vector.add_range_wrap(xabs_b[:], xs_b[:], shift=PI / 2, bound=PI,
                                         period=TWO_PI)
                nc.vector.add_range_wrap(xabs_a[:], xs_a[:], shift=PI / 2, bound=PI,
                                         period=TWO_PI)
                nc.scalar.activation(fb_c[:], xabs_b[:], AF.Sin)
                nc.scalar.activation(fa_c[:], xabs_a[:], AF.Sin)
            else:
                xabs_b = xpool.tile([128, GROUP * NB], F32, tag="xabs_b", name="xabs_b")
                xabs_a = xpool.tile([128, GROUP * NA], F32, tag="xabs_a", name="xabs_a")
                nc.scalar.activation(xabs_b[:], xs_b[:], AF.Abs)
                nc.scalar.activation(xabs_a[:], xs_a[:], AF.Abs)
                nc.scalar.activation(fb_c[:], xabs_b[:], AF.Sin, scale=-1.0,
                                     bias=half_pi[:, 0:1])
                nc.scalar.activation(fa_c[:], xabs_a[:], AF.Sin, scale=-1.0,
                                     bias=half_pi[:, 0:1])
            # scale a-side by |We_h|*c_m (fp16 in-place)
            for jl, j in enumerate(ms):
                for t in range(HT):
                    asl = slice(jl * NA + t * LQC, jl * NA + (t + 1) * LQC)
                    wcol = wc[:, t * M_TERMS + j: t * M_TERMS + j + 1]
                    nc.vector.tensor_scalar(fa_s[:, asl], fa_s[:, asl], wcol, None,
                                            ALU.mult)
                    nc.vector.tensor_scalar(fa_c[:, asl], fa_c[:, asl], wcol, None,
                                            ALU.mult)
            feats.append((fb_s, fb_c, fa_s, fa_c))

        # ---------- score matmuls: group-outer, qb interleaved ----------
        for g in range(n_groups):
            fb_s, fb_c, fa_s, fa_c = feats[g]
            for jl in range(GROUP):
                j = g * GROUP + jl
                for t in range(HT):
                    bsl = slice(jl * NB + t * LK, jl * NB + (t + 1) * LK)
                    for qb in range(QB):
                        asl = slice(jl * NA + t * LQC + qb * 128,
                                    jl * NA + t * LQC + qb * 128 + 128)
                        first = (j == 0 and t == 0)
                        last = (j == M_TERMS - 1 and t == HT - 1)
                        nc.tensor.matmul(ps_sc[qb][:], fa_s[:, asl], fb_c[:, bsl],
                                         start=first, stop=False)
                        nc.tensor.matmul(ps_sc[qb][:], fa_c[:, asl], fb_s[:, bsl],
                                         start=False, stop=last)

        for qb in range(QB):
            # ---------- softmax + context for this q-block ----------
            negmax = work.tile([128, 1], F32, tag="negmax", name="negmax")
            nc.vector.reduce_max(negmax[:], ps_sc[qb][:], axis=mybir.AxisListType.X,
                                 negate=True)
            unnorm = work.tile([128, LK], F32, tag="unnorm", name="unnorm")
            rowsum = work.tile([128, 1], F32, tag="rowsum", name="rowsum")
            nc.scalar.activation(unnorm[:], ps_sc[qb][:], AF.Exp,
                                 bias=negmax[:, 0:1], accum_out=rowsum[:])
            rinv = work.tile([128, 1], F32, tag="rinv", name="rinv")
            nc.vector.reciprocal(rinv[:], rowsum[:])
            attn16 = persist.tile([128, LK], F16, tag=f"attn{qb}", name=f"attn{qb}")
            nc.vector.tensor_scalar(attn16[:], unnorm[:], rinv[:, 0:1], None, ALU.mult)
            nc.sync.dma_start(attn_d[qb * 128:(qb + 1) * 128, :], attn16[:])

            psc = ps_proj.tile([128, LK], F32, tag="pj", name="psc")[:, :H]
            for kt in range(KT):
                pst = ps_small.tile([128, 128], F16, tag="trps16", name="atps")
                nc.tensor.transpose(pst[:], attn16[:, kt * 128:(kt + 1) * 128],
                                    ident16[:])
                at = work.tile([128, 128], F16, tag="at", name="at")
                nc.vector.tensor_copy(at[:], pst[:])
                nc.tensor.matmul(psc[:], at[:], vn16[:, kt * H:(kt + 1) * H],
                                 start=(kt == 0), stop=(kt == KT - 1))
            ctx_s = work.tile([128, H], F32, tag="ctx_s", name="ctx_s")
            nc.scalar.copy(ctx_s[:], psc[:])
            nc.sync.dma_start(ctx_d[qb * 128:(qb + 1) * 128, :], ctx_s[:])

    nc.compile()
    return nc


def _get_program():
    global _PROGRAM
    if _PROGRAM is None:
        _PROGRAM = _build_program()
    return _PROGRAM


def _make_in_maps(query, values, Wq, bq, Wk, bk, We):
    query = np.asarray(query, dtype=np.float32)
    values = np.asarray(values, dtype=np.float32)
    Wq = np.asarray(Wq, dtype=np.float32)
    Wk = np.asarray(Wk, dtype=np.float32)
    bq = np.asarray(bq, dtype=np.float32)
    bk = np.asarray(bk, dtype=np.float32)
    We = np.asarray(We, dtype=np.float32)

    # host-side prep: transposed fp16 layouts + We-derived vectors
    wqT = np.ascontiguousarray(Wq.T.astype(np.float16))
    wkT = np.ascontiguousarray(Wk.T.astype(np.float16))
    bqT = np.ascontiguousarray(bq.reshape(HT, 128).T.astype(np.float32))
    bkT = np.ascontiguousarray(bk.reshape(HT, 128).T.astype(np.float32))
    sgn = np.where(We < 0, -1.0, 1.0).astype(np.float32) * np.float32(T_SCALE)
    sgn_sc = np.ascontiguousarray(sgn.reshape(HT, 128).T)
    wabs = np.abs(We).astype(np.float32)
    wc = np.zeros((128, HT * M_TERMS), np.float32)
    for t in range(HT):
        for j in range(M_TERMS):
            wc[:, t * M_TERMS + j] = wabs[t * 128:(t + 1) * 128] * np.float32(COEF[j])
    wc = np.ascontiguousarray(wc)

    in_maps = []
    for c in range(8):
        b, qh = c // 2, c % 2
        qloc = query[b, qh * LQC:(qh + 1) * LQC]
        in_maps.append({
            "queryT": np.ascontiguousarray(qloc.T.astype(np.float16)),
            "valuesT": np.ascontiguousarray(values[b].T.astype(np.float16)),
            "values16": np.ascontiguousarray(values[b].astype(np.float16)),
            "WqT": wqT, "WkT": wkT,
            "bqT": bqT, "bkT": bkT, "sgn_sc": sgn_sc, "wc": wc,
        })
    return in_maps


def kernel(query, values, Wq, bq, Wk, bk, We, be=None, **_unused):
    in_maps = _make_in_maps(query, values, Wq, bq, Wk, bk, We)
    nc = _get_program()
    res = run_bass_kernel_spmd(nc, in_maps, list(range(8)))

    context = np.zeros((B, LQ, H), np.float32)
    attention = np.zeros((B, LQ, LK), np.float32)
    for c in range(8):
        b, qh = c // 2, c % 2
        context[b, qh * LQC:(qh + 1) * LQC] = res.results[c]["ctxout"]
        attention[b, qh * LQC:(qh + 1) * LQC] = res.results[c]["attn"]
    return context, attention


# revision 22
# speedup vs baseline: 1.0552x; 1.0552x over previous
"""Bahdanau additive attention on 8 Trainium2 NeuronCores (Bass/Tile).

Math: context, attention = softmax_k(sum_h We_h * tanh(qproj[q,h] + kproj[k,h]) + be)
per batch.  be shifts all scores equally -> softmax-invariant -> dropped.

Key trick: tanh(a+b) ~= sum_m c_m sin(w_m (a+b))
                      = sum_m c_m [sin(w_m a) cos(w_m b) + cos(w_m a) sin(w_m b)]
with quarter-wave harmonics w_m = (2m-1) pi/(2L).  This factorizes the
(Lq x Lk x H) tanh into per-side sin/cos features contracted by the Tensor
engine.  The ScalarE Sin LUT is only valid on [-pi,pi]: angles are
range-reduced exactly on the Vector engine with the magic-number rounding
trick (skipped for the lowest harmonics whose angles provably stay in
range); cos(y) = sin(pi/2 - |y|) via an ACT Abs pass + Sin(scale=-1,
bias=pi/2).  |We|*c_m folds into the a-side features; sign(We) folds into
the projections (tanh is odd).  Matmul operands are fp16 (single-pass PE;
fp32 is LOW_HIGH double-pass); accumulation is fp32 in PSUM and the
range-reduction arithmetic is fp32.  Transposed/cast operand layouts and
the tiny We-derived vectors are prepared host-side during input sharding.

Sharding: core c <- (batch b=c//2, query-half qh=c%2); each core computes a
[256 q x 512 k] attention block + [256 x 256] context block independently.
"""
from contextlib import ExitStack

import numpy as np

import concourse.bass as bass
import concourse.mybir as mybir
import concourse.tile as tile
from concourse import bacc
from concourse.bass_utils import run_bass_kernel_spmd

F32 = mybir.dt.float32
F16 = mybir.dt.float16
AF = mybir.ActivationFunctionType
ALU = mybir.AluOpType

# ---- problem shape (hardcoded per contract) ----
B, LQ, LK, H = 4, 512, 512, 256
LQC = LQ // 2          # per-core queries = 256
HT = H // 128          # h tiles = 2
KT = LK // 128         # k tiles = 4
QB = LQC // 128        # q blocks per core = 2

# ---- sine approximation of tanh ----
M_TERMS = 10
GROUP = 2              # m's per feature group
L_FIT = 6.5
MAGIC = float(np.float32(1.5 * 2 ** 23))
TWO_PI = float(np.float32(2 * np.pi))
PI = float(np.pi)
T_SCALE = 1.0 / (4.0 * L_FIT)    # turns per unit: w_base/(2*pi) = 1/(4L)
# |2pi*mp*t| stays inside [-pi,pi] for |proj|<=4.3 -> no range reduction.
FOLD_FREE = {1.0, 3.0}
# |2pi*mp*t| stays inside [-3pi,3pi] -> one add_range_wrap suffices.
WRAP_FOLD = {5.0, 7.0}
# |2pi*mp*t| stays inside [-pi/2,pi/2] -> cos needs no abs (pi/2 - y in range).
ABS_FREE = {1.0}

NA = HT * LQC   # 512  (a-side feature width per m)
NB = HT * LK    # 1024 (b-side feature width per m)


def _fit_coeffs():
    x = np.linspace(0, L_FIT, 6001)
    mult = 2 * np.arange(1, M_TERMS + 1) - 1
    w = mult * np.pi / (2 * L_FIT)
    A = np.sin(np.outer(x, w))
    c, *_ = np.linalg.lstsq(A, np.tanh(x), rcond=None)
    return [float(m) for m in mult], [float(v) for v in c]


MULT, COEF = _fit_coeffs()

_PROGRAM = None


def _build_program():
    nc = bacc.Bacc("TRN2", target_bir_lowering=False, debug=False, num_devices=8)

    # Pre-transposed fp16 matmul operands (prepared host-side):
    qT_d = nc.dram_tensor("queryT", [H, LQC], F16, kind="ExternalInput").ap()
    vT_d = nc.dram_tensor("valuesT", [H, LK], F16, kind="ExternalInput").ap()
    v16_d = nc.dram_tensor("values16", [LK, H], F16, kind="ExternalInput").ap()
    wqT_d = nc.dram_tensor("WqT", [H, H], F16, kind="ExternalInput").ap()
    wkT_d = nc.dram_tensor("WkT", [H, H], F16, kind="ExternalInput").ap()
    # Small fp32 vectors (bias / We-derived), in [128, x] on-chip layout:
    bqT_d = nc.dram_tensor("bqT", [128, HT], F32, kind="ExternalInput").ap()
    bkT_d = nc.dram_tensor("bkT", [128, HT], F32, kind="ExternalInput").ap()
    sgn_d = nc.dram_tensor("sgn_sc", [128, HT], F32, kind="ExternalInput").ap()
    wc_d = nc.dram_tensor("wc", [128, HT * M_TERMS], F32, kind="ExternalInput").ap()
    attn_d = nc.dram_tensor("attn", [LQC, LK], F16, kind="ExternalOutput").ap()
    ctx_d = nc.dram_tensor("ctxout", [LQC, H], F32, kind="ExternalOutput").ap()

    with tile.TileContext(nc) as tc, ExitStack() as ctx:
        persist = ctx.enter_context(tc.tile_pool(name="persist", bufs=1))
        work = ctx.enter_context(tc.tile_pool(name="work", bufs=2))
        xpool = ctx.enter_context(tc.tile_pool(name="xpool", bufs=2))
        fpool = ctx.enter_context(tc.tile_pool(name="fpool", bufs=1))
        ps_small = ctx.enter_context(tc.tile_pool(name="ps_small", bufs=2, space="PSUM"))
        ps_proj = ctx.enter_context(tc.tile_pool(name="ps_proj", bufs=2, space="PSUM"))
        ps_score = ctx.enter_context(tc.tile_pool(name="ps_score", bufs=1, space="PSUM"))

        from concourse.masks import make_identity
        ident16 = persist.tile([128, 128], F16, tag="ident16", name="ident16")
        make_identity(nc, ident16)
        half_pi = persist.tile([128, 1], F32, tag="half_pi", name="half_pi")
        nc.vector.memset(half_pi[:], PI / 2)

        # ---------- loads ----------
        valuesT = persist.tile([128, HT * LK], F16, tag="valuesT", name="valuesT")
        wkTt = persist.tile([128, HT * H], F16, tag="wkTt", name="wkTt")
        queryT = persist.tile([128, HT * LQC], F16, tag="queryT", name="queryT")
        wqTt = persist.tile([128, HT * H], F16, tag="wqTt", name="wqTt")
        vn16 = persist.tile([128, KT * H], F16, tag="vn16", name="vn16")
        nc.sync.dma_start(valuesT[:], vT_d.rearrange("(t p) k -> p t k", p=128))
        nc.sync.dma_start(wkTt[:], wkT_d.rearrange("(t p) o -> p t o", p=128))
        nc.sync.dma_start(queryT[:], qT_d.rearrange("(t p) i -> p t i", p=128))
        nc.sync.dma_start(wqTt[:], wqT_d.rearrange("(t p) o -> p t o", p=128))
        nc.scalar.dma_start(vn16[:], v16_d.rearrange("(i p) h -> p i h", p=128))
        bqT = persist.tile([128, HT], F32, tag="bqT", name="bqT")
        bkT = persist.tile([128, HT], F32, tag="bkT", name="bkT")
        sgn_sc = persist.tile([128, HT], F32, tag="sgn_sc", name="sgn_sc")
        wc = persist.tile([128, HT * M_TERMS], F32, tag="wc", name="wc")
        nc.scalar.dma_start(bkT[:], bkT_d)
        nc.scalar.dma_start(sgn_sc[:], sgn_d)
        nc.scalar.dma_start(bqT[:], bqT_d)
        nc.scalar.dma_start(wc[:], wc_d)

        # ---------- projections -> turn grids (fp32) ----------
        tk = persist.tile([128, NB], F32, tag="tk", name="tk")   # [t0 512 | t1 512]
        tq = persist.tile([128, NA], F32, tag="tq", name="tq")   # [t0 256 | t1 256]
        for ot in range(HT):
            psk = ps_proj.tile([128, LK], F32, tag="pj", name="psk")
            for ht in range(HT):
                nc.tensor.matmul(psk[:], wkTt[:, ht * H + ot * 128: ht * H + (ot + 1) * 128],
                                 valuesT[:, ht * LK:(ht + 1) * LK],
                                 start=(ht == 0), stop=(ht == HT - 1))
            tmp = work.tile([128, LK], F32, tag="projk", name="projk")
            nc.scalar.activation(tmp[:], psk[:], AF.Identity, bias=bkT[:, ot:ot + 1])
            nc.vector.tensor_scalar(tk[:, ot * LK:(ot + 1) * LK], tmp[:],
                                    sgn_sc[:, ot:ot + 1], None, ALU.mult)
        for ot in range(HT):
            psq = ps_proj.tile([128, LK], F32, tag="pj", name="psq")[:, :LQC]
            for ht in range(HT):
                nc.tensor.matmul(psq[:], wqTt[:, ht * H + ot * 128: ht * H + (ot + 1) * 128],
                                 queryT[:, ht * LQC:(ht + 1) * LQC],
                                 start=(ht == 0), stop=(ht == HT - 1))
            tmp = work.tile([128, LQC], F32, tag="projq", name="projq")
            nc.scalar.activation(tmp[:], psq[:], AF.Identity, bias=bqT[:, ot:ot + 1])
            nc.vector.tensor_scalar(tq[:, ot * LQC:(ot + 1) * LQC], tmp[:],
                                    sgn_sc[:, ot:ot + 1], None, ALU.mult)

        # ---------- fold helper ----------
        def fold_side(src, n, mp, xs_dst):
            """xs_dst <- 2pi*mp*src - 2pi*round(mp*src), in [-pi, pi]."""
            if mp in FOLD_FREE:
                nc.vector.tensor_scalar(xs_dst, src[:], TWO_PI * mp, None, ALU.mult)
                return
            if mp in WRAP_FOLD:
                rad = xpool.tile([128, n], F32, tag=f"rad{n}", name=f"rad{n}")
                nc.vector.tensor_scalar(rad[:], src[:], TWO_PI * mp, None, ALU.mult)
                nc.vector.add_range_wrap(xs_dst, rad[:], shift=0.0, bound=PI,
                                         period=TWO_PI)
                return
            ys = xpool.tile([128, n], F32, tag=f"ys{n}", name=f"ys{n}")
            nc.vector.tensor_scalar(ys[:], src[:], mp, MAGIC, ALU.mult, ALU.add)
            z = xpool.tile([128, n], F32, tag=f"z{n}", name=f"z{n}")
            nc.vector.tensor_scalar(z[:], ys[:], MAGIC, -TWO_PI,
                                    ALU.subtract, ALU.mult)
            nc.vector.scalar_tensor_tensor(xs_dst, in0=src[:], scalar=TWO_PI * mp,
                                           in1=z[:], op0=ALU.mult, op1=ALU.add)

        # ---------- feature groups ----------
        ps_sc = [ps_score.tile([128, LK], F32, tag=f"scores{qb}", name=f"scores{qb}")
                 for qb in range(QB)]

        n_groups = M_TERMS // GROUP
        feats = []   # per group: (fb_s, fb_c, fa_s, fa_c)
        for g in range(n_groups):
            ms = [g * GROUP + jl for jl in range(GROUP)]
            xs_b = xpool.tile([128, GROUP * NB], F32, tag="xs_b", name="xs_b")
            xs_a = xpool.tile([128, GROUP * NA], F32, tag="xs_a", name="xs_a")
            for jl, j in enumerate(ms):
                mp = MULT[j]
                fold_side(tk, NB, mp, xs_b[:, jl * NB:(jl + 1) * NB])
                fold_side(tq, NA, mp, xs_a[:, jl * NA:(jl + 1) * NA])
            all_abs_free = all(MULT[j] in ABS_FREE for j in ms)
            fb_s = fpool.tile([128, GROUP * NB], F16, tag=f"fb_s{g}", name=f"fb_s{g}")
            fb_c = fpool.tile([128, GROUP * NB], F16, tag=f"fb_c{g}", name=f"fb_c{g}")
            fa_s = fpool.tile([128, GROUP * NA], F16, tag=f"fa_s{g}", name=f"fa_s{g}")
            fa_c = fpool.tile([128, GROUP * NA], F16, tag=f"fa_c{g}", name=f"fa_c{g}")
            nc.scalar.activation(fb_s[:], xs_b[:], AF.Sin)
            nc.scalar.activation(fa_s[:], xs_a[:], AF.Sin)
            if all_abs_free:
                nc.scalar.activation(fb_c[:], xs_b[:], AF.Sin, scale=-1.0,
                                     bias=half_pi[:, 0:1])
                nc.scalar.activation(fa_c[:], xs_a[:], AF.Sin, scale=-1.0,
                                     bias=half_pi[:, 0:1])
            elif g == 2:
                # balance: this group's cos-prep on the Vector engine
                xabs_b = xpool.tile([128, GROUP * NB], F32, tag="xabs_b", name="xabs_b")
                xabs_a = xpool.tile([128, GROUP * NA], F32, tag="xabs_a", name="xabs_a")
                nc.vector.add_range_wrap(xabs_b[:], xs_b[:], shift=PI / 2, bound=PI,
                                         period=TWO_PI)
                nc.vector.add_range_wrap(xabs_a[:], xs_a[:], shift=PI / 2, bound=PI,
                                         period=TWO_PI)
                nc.scalar.activation(fb_c[:], xabs_b[:], AF.Sin)
                nc.scalar.activation(fa_c[:], xabs_a[:], AF.Sin)
            else:
                xabs_b = xpool.tile([128, GROUP * NB], F32, tag="xabs_b", name="xabs_b")
                xabs_a = xpool.tile([128, GROUP * NA], F32, tag="xabs_a", name="xabs_a")
                nc.scalar.activation(xabs_b[:], xs_b[:], AF.Abs)
                nc.scalar.activation(xabs_a[:], xs_a[:], AF.Abs)
                nc.scalar.activation(fb_c[:], xabs_b[:], AF.Sin, scale=-1.0,
                                     bias=half_pi[:, 0:1])
                nc.scalar.activation(fa_c[:], xabs_a[:], AF.Sin, scale=-1.0,
                                     bias=half_pi[:, 0:1])
            # scale a-side by |We_h|*c_m (fp16 in-place)
            for jl, j in enumerate(ms):
                for t in range(HT):
                    asl = slice(jl * NA + t * LQC, jl * NA + (t + 1) * LQC)
                    wcol = wc[:, t * M_TERMS + j: t * M_TERMS + j + 1]
                    nc.vector.tensor_scalar(fa_s[:, asl], fa_s[:, asl], wcol, None,
                                            ALU.mult)
                    nc.vector.tensor_scalar(fa_c[:, asl], fa_c[:, asl], wcol, None,
                                            ALU.mult)
            feats.append((fb_s, fb_c, fa_s, fa_c))

        # ---------- score matmuls: group-outer, qb interleaved ----------
        for g in range(n_groups):
            fb_s, fb_c, fa_s, fa_c = feats[g]
            for jl in range(GROUP):
                j = g * GROUP + jl
                for t in range(HT):
                    bsl = slice(jl * NB + t * LK, jl * NB + (t + 1) * LK)
                    for qb in range(QB):
                        asl = slice(jl * NA + t * LQC + qb * 128,
                                    jl * NA + t * LQC + qb * 128 + 128)
                        first = (j == 0 and t == 0)
                        last = (j == M_TERMS - 1 and t == HT - 1)
                        nc.tensor.matmul(ps_sc[qb][:], fa_s[:, asl], fb_c[:, bsl],
                                         start=first, stop=False)
                        nc.tensor.matmul(ps_sc[qb][:], fa_c[:, asl], fb_s[:, bsl],
                                         start=False, stop=last)

        for qb in range(QB):
            # ---------- softmax + context for this q-block ----------
            negmax = work.tile([128, 1], F32, tag="negmax", name="negmax")
            nc.vector.reduce_max(negmax[:], ps_sc[qb][:], axis=mybir.AxisListType.X,
                                 negate=True)
            unnorm = work.tile([128, LK], F32, tag="unnorm", name="unnorm")
            rowsum = work.tile([128, 1], F32, tag="rowsum", name="rowsum")
            nc.scalar.activation(unnorm[:], ps_sc[qb][:], AF.Exp,
                                 bias=negmax[:, 0:1], accum_out=rowsum[:])
            rinv = work.tile([128, 1], F32, tag="rinv", name="rinv")
            nc.vector.reciprocal(rinv[:], rowsum[:])
            attn16 = persist.tile([128, LK], F16, tag=f"attn{qb}", name=f"attn{qb}")
            nc.vector.tensor_scalar(attn16[:], unnorm[:], rinv[:, 0:1], None, ALU.mult)
            nc.sync.dma_start(attn_d[qb * 128:(qb + 1) * 128, :], attn16[:])

            psc = ps_proj.tile([128, LK], F32, tag="pj", name="psc")[:, :H]
            for kt in range(KT):
                pst = ps_small.tile([128, 128], F16, tag="trps16", name="atps")
                nc.tensor.transpose(pst[:], attn16[:, kt * 128:(kt + 1) * 128],
                                    ident16[:])
                at = work.tile([128, 128], F16, tag="at", name="at")
                nc.vector.tensor_copy(at[:], pst[:])
                nc.tensor.matmul(psc[:], at[:], vn16[:, kt * H:(kt + 1) * H],
                                 start=(kt == 0), stop=(kt == KT - 1))
            ctx_s = work.tile([128, H], F32, tag="ctx_s", name="ctx_s")
            nc.scalar.copy(ctx_s[:], psc[:])
            nc.sync.dma_start(ctx_d[qb * 128:(qb + 1) * 128, :], ctx_s[:])

    nc.compile()
    return nc


def _get_program():
    global _PROGRAM
    if _PROGRAM is None:
        _PROGRAM = _build_program()
    return _PROGRAM


def _make_in_maps(query, values, Wq, bq, Wk, bk, We):
    query = np.asarray(query, dtype=np.float32)
    values = np.asarray(values, dtype=np.float32)
    Wq = np.asarray(Wq, dtype=np.float32)
    Wk = np.asarray(Wk, dtype=np.float32)
    bq = np.asarray(bq, dtype=np.float32)
    bk = np.asarray(bk, dtype=np.float32)
    We = np.asarray(We, dtype=np.float32)

    # host-side prep: transposed fp16 layouts + We-derived vectors
    wqT = np.ascontiguousarray(Wq.T.astype(np.float16))
    wkT = np.ascontiguousarray(Wk.T.astype(np.float16))
    bqT = np.ascontiguousarray(bq.reshape(HT, 128).T.astype(np.float32))
    bkT = np.ascontiguousarray(bk.reshape(HT, 128).T.astype(np.float32))
    sgn = np.where(We < 0, -1.0, 1.0).astype(np.float32) * np.float32(T_SCALE)
    sgn_sc = np.ascontiguousarray(sgn.reshape(HT, 128).T)
    wabs = np.abs(We).astype(np.float32)
    wc = np.zeros((128, HT * M_TERMS), np.float32)
    for t in range(HT):
        for j in range(M_TERMS):
            wc[:, t * M_TERMS + j] = wabs[t * 128:(t + 1) * 128] * np.float32(COEF[j])
    wc = np.ascontiguousarray(wc)

    in_maps = []
    for c in range(8):
        b, qh = c // 2, c % 2
        qloc = query[b, qh * LQC:(qh + 1) * LQC]
        in_maps.append({
            "queryT": np.ascontiguousarray(qloc.T.astype(np.float16)),
            "valuesT": np.ascontiguousarray(values[b].T.astype(np.float16)),
            "values16": np.ascontiguousarray(values[b].astype(np.float16)),
            "WqT": wqT, "WkT": wkT,
            "bqT": bqT, "bkT": bkT, "sgn_sc": sgn_sc, "wc": wc,
        })
    return in_maps


def kernel(query, values, Wq, bq, Wk, bk, We, be=None, **_unused):
    in_maps = _make_in_maps(query, values, Wq, bq, Wk, bk, We)
    nc = _get_program()
    res = run_bass_kernel_spmd(nc, in_maps, list(range(8)))

    context = np.zeros((B, LQ, H), np.float32)
    attention = np.zeros((B, LQ, LK), np.float32)
    for c in range(8):
        b, qh = c // 2, c % 2
        context[b, qh * LQC:(qh + 1) * LQC] = res.results[c]["ctxout"]
        attention[b, qh * LQC:(qh + 1) * LQC] = res.results[c]["attn"]
    return context, attention
